# revision 4
# baseline (speedup 1.0000x reference)
"""ConfSMoE Trainium2 kernel (expert-parallel across 8 NeuronCores).

Strategy
--------
B,S,D,E,H = 8,512,512,8,2048; top-2-of-8 confidence-weighted MoE.
- Core i owns expert i and token shard i (batch i).
- Each core: LayerNorm(own 512 tokens) in fp32, transposes its shard
  (PE transpose) to get xn^T, computes router logits in fp32 for its
  shard, top-2 mask + renormalized confidence via DVE Max8.
- AllGather the bf16 xn^T shards and the fp32 gate weights.
- Dense expert FFN for all 4096 tokens through its own expert in bf16:
  h^T = gelu_tanh(W1_e^T-tiles @ xn^T + b1), o = h^T-tiles @ W2_e + b2
  (b2 added as a K=1 rank-1 matmul into the same PSUM accumulation).
  The top-2 gate g_e[token] multiplies o as a per-partition scalar.
- Partial outputs ReduceScattered in 4 chunks of 1024 tokens; each core
  gets 128 tokens per chunk; residual-add + final LayerNorm on those.
- Host reassembles: out[1024*c + 128*i + t] = core_i.out[c, t].
"""

import numpy as np

import concourse.bass as bass
import concourse.mybir as mybir
import concourse.tile as tile
from concourse import bacc
from concourse.bass_utils import run_bass_kernel_spmd
from concourse.masks import make_identity

B, S, D, E, H = 8, 512, 512, 8, 2048
N_CORES = 8
T = B * S            # 4096 tokens
TS = S               # tokens per shard
EPS = 1e-5
P = 128              # partitions
KD = D // P          # 4  D-tiles
KH = H // P          # 16 H-tiles
NT = TS // P         # 4  token tiles per shard
NCHUNK = 4           # ReduceScatter chunks (2 batches each)

FP32 = mybir.dt.float32
BF16 = mybir.dt.bfloat16
AF = mybir.ActivationFunctionType
ALU = mybir.AluOpType
AX = mybir.AxisListType



def _bc(dram_param, p, n):
    """[n]-shaped DRAM tensor broadcast to [p, n] via a step-0 partition dim."""
    a = dram_param.ap()
    return bass.AP(tensor=a.tensor, offset=a.offset, ap=[[0, p]] + list(a.ap))


def build():
    nc = bacc.Bacc("TRN2", target_bir_lowering=False, debug=False,
                   num_devices=N_CORES)

    # ---------------- I/O ----------------
    x_sh = nc.declare_dram_parameter("x_sh", [TS, D], FP32, isOutput=False)
    x_res = nc.declare_dram_parameter("x_res", [NCHUNK, P, D], FP32, isOutput=False)
    Wg_d = nc.declare_dram_parameter("Wg", [D, E], FP32, isOutput=False)
    W1_d = nc.declare_dram_parameter("W1e", [D, H], FP32, isOutput=False)
    b1_d = nc.declare_dram_parameter("b1e", [H], FP32, isOutput=False)
    W2_d = nc.declare_dram_parameter("W2e", [H, D], FP32, isOutput=False)
    b2_d = nc.declare_dram_parameter("b2e", [D], FP32, isOutput=False)
    ln_g_d = nc.declare_dram_parameter("ln_g", [D], FP32, isOutput=False)
    ln_b_d = nc.declare_dram_parameter("ln_b", [D], FP32, isOutput=False)
    out_g_d = nc.declare_dram_parameter("out_g", [D], FP32, isOutput=False)
    out_b_d = nc.declare_dram_parameter("out_b", [D], FP32, isOutput=False)
    sel_d = nc.declare_dram_parameter("sel", [E], FP32, isOutput=False)
    out_d = nc.declare_dram_parameter("out", [NCHUNK, P, D], FP32, isOutput=True)

    # ------------- internal DRAM (collectives) -------------
    xnT_sh_dram = nc.dram_tensor("xnT_sh", [D, TS], BF16)
    xnT_full = nc.dram_tensor("xnT_full", [N_CORES, D, TS], BF16,
                              addr_space="Shared")
    w_sh_dram = nc.dram_tensor("w_sh", [TS, E], FP32)
    w_full = nc.dram_tensor("w_full", [N_CORES, TS, E], FP32,
                            addr_space="Shared")
    partial_dram = [nc.dram_tensor(f"partial{c}", [2 * S, D], FP32)
                    for c in range(NCHUNK)]
    rs_out_dram = [nc.dram_tensor(f"rs_out{c}", [P, D], FP32)
                   for c in range(NCHUNK)]

    rg = [list(range(N_CORES))]

    with tile.TileContext(nc) as tc:
        with (
            tc.tile_pool(name="params", bufs=1) as ppool,
            tc.tile_pool(name="wts", bufs=1) as wpool,
            tc.tile_pool(name="xn", bufs=1) as xnpool,
            tc.tile_pool(name="route", bufs=2) as rpool,
            tc.tile_pool(name="chunk", bufs=2) as cpool,
            tc.tile_pool(name="hT", bufs=32) as hpool,
            tc.tile_pool(name="fin", bufs=2) as fpool,
            tc.tile_pool(name="ps_tr", bufs=2, space="PSUM") as ps_tr,
            tc.tile_pool(name="ps_lg", bufs=1, space="PSUM") as ps_lg,
            tc.tile_pool(name="ps_h", bufs=2, space="PSUM") as ps_h,
            tc.tile_pool(name="ps_o", bufs=2, space="PSUM") as ps_o,
        ):
            # ---------------- constants / params ----------------
            G1 = ppool.tile([P, D], FP32, tag="G1")
            B1t = ppool.tile([P, D], FP32, tag="B1t")
            OG = ppool.tile([P, D], FP32, tag="OG")
            OB = ppool.tile([P, D], FP32, tag="OB")
            nc.gpsimd.dma_start(out=G1, in_=_bc(ln_g_d, P, D))
            nc.gpsimd.dma_start(out=B1t, in_=_bc(ln_b_d, P, D))
            nc.gpsimd.dma_start(out=OG, in_=_bc(out_g_d, P, D))
            nc.gpsimd.dma_start(out=OB, in_=_bc(out_b_d, P, D))

            eps_t = ppool.tile([P, 1], FP32, tag="eps")
            nc.vector.memset(eps_t, EPS)

            b1_sb = ppool.tile([P, KH], FP32, tag="b1_sb")
            nc.sync.dma_start(out=b1_sb,
                              in_=b1_d.ap().rearrange("(m p) -> p m", p=P))

            b2_f = ppool.tile([1, D], FP32, tag="b2_f")
            nc.sync.dma_start(out=b2_f, in_=_bc(b2_d, 1, D))
            b2_bf = ppool.tile([1, D], BF16, tag="b2_bf")
            nc.vector.tensor_copy(b2_bf, b2_f)

            ones_bf = ppool.tile([1, P], BF16, tag="ones_bf")
            nc.vector.memset(ones_bf, 1.0)

            sel_bc = ppool.tile([P, E], FP32, tag="sel_bc")
            nc.gpsimd.dma_start(out=sel_bc, in_=_bc(sel_d, P, E))

            ident = ppool.tile([P, P], FP32, tag="ident")
            make_identity(nc, ident)

            # Wg as 4 K-tiles [128, 8] in one [128, 32] tile
            wg_sb = ppool.tile([P, KD, E], FP32, tag="wg_sb")
            nc.sync.dma_start(out=wg_sb,
                              in_=Wg_d.ap().rearrange("(k p) e -> p k e", p=P))

            # ---------------- weights: load fp32, cast to bf16 ----------------
            w1_bf = []
            for k in range(KD):
                stage = wpool.tile([P, H], FP32, tag="w1_stage", bufs=2)
                nc.sync.dma_start(out=stage, in_=W1_d[k * P:(k + 1) * P, :])
                wbf = wpool.tile([P, H], BF16, tag=f"w1_bf{k}")
                nc.vector.tensor_copy(wbf, stage)
                w1_bf.append(wbf)
            w2_bf = []
            for m in range(KH):
                stage = wpool.tile([P, D], FP32, tag="w2_stage", bufs=2)
                nc.sync.dma_start(out=stage, in_=W2_d[m * P:(m + 1) * P, :])
                wbf = wpool.tile([P, D], BF16, tag=f"w2_bf{m}")
                nc.vector.tensor_copy(wbf, stage)
                w2_bf.append(wbf)

            # ---------------- LayerNorm on own shard (fp32) ----------------
            xn_t = []
            for t in range(NT):
                xt = xnpool.tile([P, D], FP32, tag=f"xn{t}")
                nc.sync.dma_start(out=xt, in_=x_sh[t * P:(t + 1) * P, :])
                stats = rpool.tile([P, 6], FP32, tag="stats")
                nc.vector.bn_stats(out=stats, in_=xt)
                mv = rpool.tile([P, 2], FP32, tag="mv")
                nc.vector.bn_aggr(out=mv, in_=stats)
                mean = mv[:, 0:1]
                var = mv[:, 1:2]
                sd = rpool.tile([P, 1], FP32, tag="sd")
                nc.scalar.activation(out=sd, in_=var, func=AF.Sqrt,
                                     bias=eps_t, scale=1.0)
                rstd = rpool.tile([P, 1], FP32, tag="rstd")
                nc.vector.reciprocal(out=rstd, in_=sd)
                nc.vector.tensor_scalar(out=xt, in0=xt, scalar1=mean,
                                        scalar2=rstd, op0=ALU.subtract,
                                        op1=ALU.mult)
                nc.vector.tensor_mul(out=xt, in0=xt, in1=G1)
                nc.vector.tensor_add(out=xt, in0=xt, in1=B1t)
                xn_t.append(xt)

            # ---------------- shard transpose (PE) ----------------
            # xnT_f32[d] : [128 D-rows, 512 tokens] fp32
            xnT_f32 = []
            for d in range(KD):
                xd = xnpool.tile([P, TS], FP32, tag=f"xnT{d}")
                xnT_f32.append(xd)
            for t in range(NT):
                for d in range(KD):
                    ptr = ps_tr.tile([P, P], FP32, tag="tr")
                    nc.tensor.transpose(ptr, xn_t[t][:, d * P:(d + 1) * P],
                                        ident)
                    nc.vector.tensor_copy(xnT_f32[d][:, t * P:(t + 1) * P],
                                          ptr)
            # bf16 copy of the shard, to DRAM, then AllGather
            for d in range(KD):
                xbf = xnpool.tile([P, TS], BF16, tag=f"xnTbf{d}")
                nc.vector.tensor_copy(xbf, xnT_f32[d])
                nc.sync.dma_start(out=xnT_sh_dram[d * P:(d + 1) * P, :],
                                  in_=xbf)
            nc.gpsimd.collective_compute(
                "AllGather", ALU.bypass, replica_groups=rg,
                ins=[xnT_sh_dram.ap()], outs=[xnT_full.ap()])

            # ---------------- router (fp32, own shard) ----------------
            for t in range(NT):
                plg = ps_lg.tile([P, E], FP32, tag="lg")
                for d in range(KD):
                    nc.tensor.matmul(plg,
                                     xnT_f32[d][:, t * P:(t + 1) * P],
                                     wg_sb[:, d, :],
                                     start=(d == 0), stop=(d == KD - 1))
                lg = rpool.tile([P, E], FP32, tag="lg_sb")
                nc.vector.tensor_copy(lg, plg)
                mx = rpool.tile([P, 8], FP32, tag="mx")
                nc.vector.max(out=mx, in_=lg)
                neg_m1 = rpool.tile([P, 1], FP32, tag="neg_m1")
                nc.vector.tensor_scalar_mul(neg_m1, mx[:, 0:1], -1.0)
                expl = rpool.tile([P, E], FP32, tag="expl")
                nc.scalar.activation(out=expl, in_=lg, func=AF.Exp,
                                     bias=neg_m1, scale=1.0)
                mask = rpool.tile([P, E], FP32, tag="mask")
                nc.vector.tensor_scalar(out=mask, in0=lg, scalar1=mx[:, 1:2],
                                        scalar2=None, op0=ALU.is_ge)
                nc.vector.tensor_mul(out=expl, in0=expl, in1=mask)
                den = rpool.tile([P, 1], FP32, tag="den")
                nc.vector.reduce_sum(out=den, in_=expl, axis=AX.X)
                rec = rpool.tile([P, 1], FP32, tag="rec")
                nc.vector.reciprocal(out=rec, in_=den)
                wgt = rpool.tile([P, E], FP32, tag="wgt")
                nc.vector.tensor_scalar_mul(wgt, expl, rec)
                nc.sync.dma_start(out=w_sh_dram[t * P:(t + 1) * P, :], in_=wgt)
            nc.gpsimd.collective_compute(
                "AllGather", ALU.bypass, replica_groups=rg,
                ins=[w_sh_dram.ap()], outs=[w_full.ap()])

            # all gates [4096, 8] -> [128, 32, 8] (token tile g on free dim)
            w_sb = ppool.tile([P, T // P, E], FP32, tag="w_sb")
            nc.sync.dma_start(
                out=w_sb,
                in_=w_full.ap().rearrange("r (g p) e -> p (r g) e", p=P))

            # ---------------- expert FFN over all tokens ----------------
            for r in range(B):  # token chunk = batch r = shard of core r
                xr = []
                for d in range(KD):
                    xt = cpool.tile([P, TS], BF16, tag=f"xr{d}")
                    nc.sync.dma_start(out=xt,
                                      in_=xnT_full[r, d * P:(d + 1) * P, :])
                    xr.append(xt)
                hts = []
                for m in range(KH):
                    ph = ps_h.tile([P, TS], FP32, tag="ph")
                    for k in range(KD):
                        nc.tensor.matmul(ph,
                                         w1_bf[k][:, m * P:(m + 1) * P],
                                         xr[k],
                                         start=(k == 0), stop=(k == KD - 1))
                    ht = hpool.tile([P, TS], BF16, tag="ht")
                    nc.scalar.activation(out=ht, in_=ph,
                                         func=AF.Gelu_apprx_tanh,
                                         bias=b1_sb[:, m:m + 1], scale=1.0)
                    hts.append(ht)
                for tt in range(NT):
                    po = ps_o.tile([P, D], FP32, tag="po")
                    for m in range(KH):
                        nc.tensor.matmul(po,
                                         hts[m][:, tt * P:(tt + 1) * P],
                                         w2_bf[m],
                                         start=(m == 0), stop=False)
                    nc.tensor.matmul(po, ones_bf, b2_bf,
                                     start=False, stop=True)
                    gidx = r * NT + tt
                    gtmp = cpool.tile([P, E], FP32, tag="gtmp")
                    nc.vector.tensor_mul(out=gtmp, in0=w_sb[:, gidx, :],
                                         in1=sel_bc)
                    gt = cpool.tile([P, 1], FP32, tag="gt")
                    nc.vector.reduce_sum(out=gt, in_=gtmp, axis=AX.X)
                    part = cpool.tile([P, D], FP32, tag="part")
                    nc.vector.tensor_scalar_mul(part, po, gt)
                    c = r // 2
                    row0 = (r % 2) * S + tt * P
                    nc.sync.dma_start(
                        out=partial_dram[c][row0:row0 + P, :], in_=part)
                if r % 2 == 1:
                    c = r // 2
                    nc.gpsimd.collective_compute(
                        "ReduceScatter", ALU.add, replica_groups=rg,
                        ins=[partial_dram[c].ap()],
                        outs=[rs_out_dram[c].ap()])

            # ---------------- residual + final LayerNorm ----------------
            for c in range(NCHUNK):
                y = fpool.tile([P, D], FP32, tag="y")
                nc.sync.dma_start(out=y, in_=rs_out_dram[c].ap())
                xres = fpool.tile([P, D], FP32, tag="xres")
                nc.sync.dma_start(out=xres, in_=x_res[c, :, :])
                nc.vector.tensor_add(out=y, in0=y, in1=xres)
                stats = fpool.tile([P, 6], FP32, tag="fstats")
                nc.vector.bn_stats(out=stats, in_=y)
                mv = fpool.tile([P, 2], FP32, tag="fmv")
                nc.vector.bn_aggr(out=mv, in_=stats)
                sd = fpool.tile([P, 1], FP32, tag="fsd")
                nc.scalar.activation(out=sd, in_=mv[:, 1:2], func=AF.Sqrt,
                                     bias=eps_t, scale=1.0)
                rstd = fpool.tile([P, 1], FP32, tag="frstd")
                nc.vector.reciprocal(out=rstd, in_=sd)
                nc.vector.tensor_scalar(out=y, in0=y, scalar1=mv[:, 0:1],
                                        scalar2=rstd, op0=ALU.subtract,
                                        op1=ALU.mult)
                nc.vector.tensor_mul(out=y, in0=y, in1=OG)
                nc.vector.tensor_add(out=y, in0=y, in1=OB)
                nc.sync.dma_start(out=out_d[c, :, :], in_=y)

    nc.finalize()
    return nc


_NC_CACHE = None


def _get_nc():
    global _NC_CACHE
    if _NC_CACHE is None:
        _NC_CACHE = build()
    return _NC_CACHE


def kernel(x, Wg, W1, b1, W2, b2, ln_g, ln_b, out_g, out_b, **_run_kwargs):
    x = np.ascontiguousarray(x, dtype=np.float32)
    xf = x.reshape(T, D)
    nc = _get_nc()
    in_maps = []
    for i in range(N_CORES):
        sel = np.zeros((E,), dtype=np.float32)
        sel[i] = 1.0
        x_res = np.stack([xf[1024 * c + P * i: 1024 * c + P * (i + 1)]
                          for c in range(NCHUNK)])
        in_maps.append({
            "x_sh": np.ascontiguousarray(x[i]),
            "x_res": np.ascontiguousarray(x_res),
            "Wg": np.ascontiguousarray(Wg, dtype=np.float32),
            "W1e": np.ascontiguousarray(W1[i], dtype=np.float32),
            "b1e": np.ascontiguousarray(b1[i], dtype=np.float32),
            "W2e": np.ascontiguousarray(W2[i], dtype=np.float32),
            "b2e": np.ascontiguousarray(b2[i], dtype=np.float32),
            "ln_g": np.ascontiguousarray(ln_g, dtype=np.float32),
            "ln_b": np.ascontiguousarray(ln_b, dtype=np.float32),
            "out_g": np.ascontiguousarray(out_g, dtype=np.float32),
            "out_b": np.ascontiguousarray(out_b, dtype=np.float32),
            "sel": sel,
        })
    res = run_bass_kernel_spmd(nc, in_maps, list(range(N_CORES)),
                               **_run_kwargs)
    out = np.empty((T, D), dtype=np.float32)
    for i in range(N_CORES):
        oc = res.results[i]["out"]  # [NCHUNK, P, D]
        for c in range(NCHUNK):
            out[1024 * c + P * i: 1024 * c + P * (i + 1)] = oc[c]
    kernel.last_results = res
    return out.reshape(B, S, D)


# revision 13
# speedup vs baseline: 1.2135x; 1.2135x over previous
"""ConfSMoE Trainium2 kernel (expert-parallel across 8 NeuronCores).

Strategy
--------
B,S,D,E,H = 8,512,512,8,2048; top-2-of-8 confidence-weighted MoE.
- Core i owns expert i and token shard i (batch i).
- Each core: LayerNorm(own 512 tokens) in fp32, transposes its shard
  (PE transpose) to get xn^T, computes router logits in fp32 for its
  shard, top-2 mask + renormalized confidence via DVE Max8.
- AllGather the bf16 xn^T shards and the fp32 gate weights.
- Dense expert FFN for all 4096 tokens through its own expert in bf16:
  h^T = gelu_tanh(W1_e^T-tiles @ xn^T + b1), o = h^T-tiles @ W2_e + b2
  (b2 added as a K=1 rank-1 matmul into the same PSUM accumulation).
  The top-2 gate g_e[token] multiplies o as a per-partition scalar.
- Partial outputs ReduceScattered in 4 chunks of 1024 tokens; each core
  gets 128 tokens per chunk; residual-add + final LayerNorm on those.
- Host reassembles: out[1024*c + 128*i + t] = core_i.out[c, t].
"""

import numpy as np

import concourse.bass as bass
import concourse.mybir as mybir
import concourse.tile as tile
from concourse import bacc
from concourse.bass_utils import run_bass_kernel_spmd
from concourse.masks import make_identity

B, S, D, E, H = 8, 512, 512, 8, 2048
N_CORES = 8
T = B * S            # 4096 tokens
TS = S               # tokens per shard
EPS = 1e-5
P = 128              # partitions
KD = D // P          # 4  D-tiles
KH = H // P          # 16 H-tiles
NT = TS // P         # 4  token tiles per shard
NCHUNK = 8           # ReduceScatter chunks (1 batch each)
NLOCAL = 2           # leading batches recomputed locally (skip AllGather dep)

FP32 = mybir.dt.float32
BF16 = mybir.dt.bfloat16
AF = mybir.ActivationFunctionType
ALU = mybir.AluOpType
AX = mybir.AxisListType



def _bc(dram_param, p, n):
    """[n]-shaped DRAM tensor broadcast to [p, n] via a step-0 partition dim."""
    a = dram_param.ap()
    return bass.AP(tensor=a.tensor, offset=a.offset, ap=[[0, p]] + list(a.ap))


def build(ln_g1=False, ln_b0=False, out_g1=False, out_b0=False, b2_0=False):
    nc = bacc.Bacc("TRN2", target_bir_lowering=False, debug=False,
                   num_devices=N_CORES)

    # ---------------- I/O ----------------
    x_sh = nc.declare_dram_parameter("x_sh", [TS, D], FP32, isOutput=False)
    x_loc = nc.declare_dram_parameter("x_loc", [NLOCAL, TS, D], FP32, isOutput=False)
    x_res = nc.declare_dram_parameter("x_res", [NCHUNK, S // N_CORES, D], FP32, isOutput=False)
    Wg_d = nc.declare_dram_parameter("Wg", [D, E], FP32, isOutput=False)
    W1_d = nc.declare_dram_parameter("W1e", [D, H], FP32, isOutput=False)
    b1_d = nc.declare_dram_parameter("b1e", [H], FP32, isOutput=False)
    W2_d = nc.declare_dram_parameter("W2e", [H, D], FP32, isOutput=False)
    b2_d = nc.declare_dram_parameter("b2e", [D], FP32, isOutput=False)
    ln_g_d = nc.declare_dram_parameter("ln_g", [D], FP32, isOutput=False)
    ln_b_d = nc.declare_dram_parameter("ln_b", [D], FP32, isOutput=False)
    out_g_d = nc.declare_dram_parameter("out_g", [D], FP32, isOutput=False)
    out_b_d = nc.declare_dram_parameter("out_b", [D], FP32, isOutput=False)
    sel_d = nc.declare_dram_parameter("sel", [E], FP32, isOutput=False)
    out_d = nc.declare_dram_parameter("out", [NCHUNK, S // N_CORES, D], FP32, isOutput=True)

    # ------------- internal DRAM (collectives) -------------
    xnT_sh_dram = nc.dram_tensor("xnT_sh", [D, TS], BF16)
    xnT_full = nc.dram_tensor("xnT_full", [N_CORES, D, TS], BF16,
                              addr_space="Shared")
    w_sh_dram = nc.dram_tensor("w_sh", [TS, E], FP32)
    w_full = nc.dram_tensor("w_full", [N_CORES, TS, E], FP32,
                            addr_space="Shared")
    partial_dram = [nc.dram_tensor(f"partial{c}", [S, D], FP32)
                    for c in range(NCHUNK)]
    rs_out_dram = [nc.dram_tensor(f"rs_out{c}", [S // N_CORES, D], FP32)
                   for c in range(NCHUNK)]

    rg = [list(range(N_CORES))]

    with tile.TileContext(nc) as tc:
        with (
            tc.tile_pool(name="params", bufs=1) as ppool,
            tc.tile_pool(name="wts", bufs=1) as wpool,
            tc.tile_pool(name="xn", bufs=1) as xnpool,
            tc.tile_pool(name="route", bufs=2) as rpool,
            tc.tile_pool(name="chunk", bufs=2) as cpool,
            tc.tile_pool(name="hT", bufs=32) as hpool,
            tc.tile_pool(name="fin", bufs=2) as fpool,
            tc.tile_pool(name="ps_tr", bufs=2, space="PSUM") as ps_tr,
            tc.tile_pool(name="ps_lg", bufs=1, space="PSUM") as ps_lg,
            tc.tile_pool(name="ps_h", bufs=3, space="PSUM") as ps_h,
            tc.tile_pool(name="ps_o", bufs=2, space="PSUM") as ps_o,
        ):
            # ---------------- constants / params ----------------
            G1 = B1t = OG = OB = None
            if not ln_g1:
                G1 = ppool.tile([P, D], FP32, tag="G1")
                nc.gpsimd.dma_start(out=G1, in_=_bc(ln_g_d, P, D))
            if not ln_b0:
                B1t = ppool.tile([P, D], FP32, tag="B1t")
                nc.gpsimd.dma_start(out=B1t, in_=_bc(ln_b_d, P, D))
            if not out_g1:
                OG = ppool.tile([P, D], FP32, tag="OG")
                nc.gpsimd.dma_start(out=OG, in_=_bc(out_g_d, P, D))
            if not out_b0:
                OB = ppool.tile([P, D], FP32, tag="OB")
                nc.gpsimd.dma_start(out=OB, in_=_bc(out_b_d, P, D))

            eps_t = ppool.tile([P, 1], FP32, tag="eps")
            nc.vector.memset(eps_t, EPS)

            b1_sb = ppool.tile([P, KH], FP32, tag="b1_sb")
            nc.sync.dma_start(out=b1_sb,
                              in_=b1_d.ap().rearrange("(m p) -> p m", p=P))

            if not b2_0:
                B2 = ppool.tile([P, D], FP32, tag="B2")
                nc.gpsimd.dma_start(out=B2, in_=_bc(b2_d, P, D))

            sel_bc = ppool.tile([P, E], FP32, tag="sel_bc")
            nc.gpsimd.dma_start(out=sel_bc, in_=_bc(sel_d, P, E))

            ident = ppool.tile([P, P], FP32, tag="ident")
            make_identity(nc, ident)

            # Wg as 4 K-tiles [128, 8] in one [128, 32] tile
            wg_sb = ppool.tile([P, KD, E], FP32, tag="wg_sb")
            nc.sync.dma_start(out=wg_sb,
                              in_=Wg_d.ap().rearrange("(k p) e -> p k e", p=P))

            # ---------------- LayerNorm on own shard (fp32) ----------------
            xn_t = []
            for t in range(NT):
                xt = xnpool.tile([P, D], FP32, tag=f"xn{t}")
                nc.sync.dma_start(out=xt, in_=x_sh[t * P:(t + 1) * P, :])
                stats = rpool.tile([P, 6], FP32, tag="stats")
                nc.vector.bn_stats(out=stats, in_=xt)
                mv = rpool.tile([P, 2], FP32, tag="mv")
                nc.vector.bn_aggr(out=mv, in_=stats)
                mean = mv[:, 0:1]
                var = mv[:, 1:2]
                sd = rpool.tile([P, 1], FP32, tag="sd")
                nc.scalar.activation(out=sd, in_=var, func=AF.Sqrt,
                                     bias=eps_t, scale=1.0)
                rstd = rpool.tile([P, 1], FP32, tag="rstd")
                nc.vector.reciprocal(out=rstd, in_=sd)
                nc.vector.tensor_scalar(out=xt, in0=xt, scalar1=mean,
                                        scalar2=rstd, op0=ALU.subtract,
                                        op1=ALU.mult)
                if not ln_g1:
                    nc.vector.tensor_mul(out=xt, in0=xt, in1=G1)
                if not ln_b0:
                    nc.vector.tensor_add(out=xt, in0=xt, in1=B1t)
                xn_t.append(xt)

            # ---------------- shard transpose (PE) ----------------
            # xnT_f32[d] : [128 D-rows, 512 tokens] fp32
            xnT_f32 = []
            for d in range(KD):
                xd = xnpool.tile([P, TS], FP32, tag=f"xnT{d}")
                xnT_f32.append(xd)
            for t in range(NT):
                for d in range(KD):
                    ptr = ps_tr.tile([P, P], FP32, tag="tr")
                    nc.tensor.transpose(ptr, xn_t[t][:, d * P:(d + 1) * P],
                                        ident)
                    nc.vector.tensor_copy(xnT_f32[d][:, t * P:(t + 1) * P],
                                          ptr)
            # bf16 copy of the shard, to DRAM, then AllGather
            for d in range(KD):
                xbf = xnpool.tile([P, TS], BF16, tag=f"xnTbf{d}")
                nc.vector.tensor_copy(xbf, xnT_f32[d])
                nc.sync.dma_start(out=xnT_sh_dram[d * P:(d + 1) * P, :],
                                  in_=xbf)
            nc.gpsimd.collective_compute(
                "AllGather", ALU.bypass, replica_groups=rg,
                ins=[xnT_sh_dram.ap()], outs=[xnT_full.ap()])

            # ---- local recompute of leading chunks (avoids AG on critical path)
            xnT_loc = {}
            for r in range(NLOCAL):
                loc_tiles = []
                for t in range(NT):
                    xt = xnpool.tile([P, D], FP32, tag=f"xloc{r}_{t}")
                    nc.sync.dma_start(out=xt, in_=x_loc[r, t * P:(t + 1) * P, :])
                    stats = rpool.tile([P, 6], FP32, tag="stats")
                    nc.vector.bn_stats(out=stats, in_=xt)
                    mv = rpool.tile([P, 2], FP32, tag="mv")
                    nc.vector.bn_aggr(out=mv, in_=stats)
                    sd = rpool.tile([P, 1], FP32, tag="sd")
                    nc.scalar.activation(out=sd, in_=mv[:, 1:2], func=AF.Sqrt,
                                         bias=eps_t, scale=1.0)
                    rstd = rpool.tile([P, 1], FP32, tag="rstd")
                    nc.vector.reciprocal(out=rstd, in_=sd)
                    nc.vector.tensor_scalar(out=xt, in0=xt, scalar1=mv[:, 0:1],
                                            scalar2=rstd, op0=ALU.subtract,
                                            op1=ALU.mult)
                    if not ln_g1:
                        nc.vector.tensor_mul(out=xt, in0=xt, in1=G1)
                    if not ln_b0:
                        nc.vector.tensor_add(out=xt, in0=xt, in1=B1t)
                    loc_tiles.append(xt)
                xl = []
                for d in range(KD):
                    xd = xnpool.tile([P, TS], BF16, tag=f"xnTloc{r}_{d}")
                    xl.append(xd)
                for t in range(NT):
                    for d in range(KD):
                        ptr = ps_tr.tile([P, P], FP32, tag="tr")
                        nc.tensor.transpose(
                            ptr, loc_tiles[t][:, d * P:(d + 1) * P], ident)
                        nc.vector.tensor_copy(
                            xl[d][:, t * P:(t + 1) * P], ptr)
                xnT_loc[r] = xl

            # ---------------- router (fp32, own shard) ----------------
            for t in range(NT):
                plg = ps_lg.tile([P, E], FP32, tag="lg")
                for d in range(KD):
                    nc.tensor.matmul(plg,
                                     xnT_f32[d][:, t * P:(t + 1) * P],
                                     wg_sb[:, d, :],
                                     start=(d == 0), stop=(d == KD - 1))
                lg = rpool.tile([P, E], FP32, tag="lg_sb")
                nc.vector.tensor_copy(lg, plg)
                mx = rpool.tile([P, 8], FP32, tag="mx")
                nc.vector.max(out=mx, in_=lg)
                neg_m1 = rpool.tile([P, 1], FP32, tag="neg_m1")
                nc.vector.tensor_scalar_mul(neg_m1, mx[:, 0:1], -1.0)
                expl = rpool.tile([P, E], FP32, tag="expl")
                nc.scalar.activation(out=expl, in_=lg, func=AF.Exp,
                                     bias=neg_m1, scale=1.0)
                mask = rpool.tile([P, E], FP32, tag="mask")
                nc.vector.tensor_scalar(out=mask, in0=lg, scalar1=mx[:, 1:2],
                                        scalar2=None, op0=ALU.is_ge)
                nc.vector.tensor_mul(out=expl, in0=expl, in1=mask)
                den = rpool.tile([P, 1], FP32, tag="den")
                nc.vector.reduce_sum(out=den, in_=expl, axis=AX.X)
                rec = rpool.tile([P, 1], FP32, tag="rec")
                nc.vector.reciprocal(out=rec, in_=den)
                wgt = rpool.tile([P, E], FP32, tag="wgt")
                nc.vector.tensor_scalar_mul(wgt, expl, rec)
                nc.sync.dma_start(out=w_sh_dram[t * P:(t + 1) * P, :], in_=wgt)
            nc.gpsimd.collective_compute(
                "AllGather", ALU.bypass, replica_groups=rg,
                ins=[w_sh_dram.ap()], outs=[w_full.ap()])

            # ---------------- weights: load fp32, cast to bf16 ----------------
            w1_bf = []
            for k in range(KD):
                stage = wpool.tile([P, H], FP32, tag="w1_stage", bufs=2)
                nc.sync.dma_start(out=stage, in_=W1_d[k * P:(k + 1) * P, :])
                wbf = wpool.tile([P, H], BF16, tag=f"w1_bf{k}")
                nc.scalar.copy(out=wbf, in_=stage)
                w1_bf.append(wbf)
            w2_bf = []
            for m in range(KH):
                stage = wpool.tile([P, D], FP32, tag="w2_stage", bufs=2)
                nc.sync.dma_start(out=stage, in_=W2_d[m * P:(m + 1) * P, :])
                wbf = wpool.tile([P, D], BF16, tag=f"w2_bf{m}")
                nc.scalar.copy(out=wbf, in_=stage)
                w2_bf.append(wbf)

            # all gates [4096, 8] -> [128, 32, 8] (token tile g on free dim)
            w_sb = ppool.tile([P, T // P, E], FP32, tag="w_sb")
            nc.sync.dma_start(
                out=w_sb,
                in_=w_full.ap().rearrange("r (g p) e -> p (r g) e", p=P))

            # ---------------- expert FFN over all tokens ----------------
            for r in range(B):  # token chunk = batch r = shard of core r
                if r < NLOCAL:
                    xr = xnT_loc[r]
                else:
                    xr = []
                    for d in range(KD):
                        xt = cpool.tile([P, TS], BF16, tag=f"xr{d}")
                        nc.sync.dma_start(out=xt,
                                          in_=xnT_full[r, d * P:(d + 1) * P, :])
                        xr.append(xt)
                hts = []
                for m in range(KH):
                    ph = ps_h.tile([P, TS], FP32, tag="ph")
                    for k in range(KD):
                        nc.tensor.matmul(ph,
                                         w1_bf[k][:, m * P:(m + 1) * P],
                                         xr[k],
                                         start=(k == 0), stop=(k == KD - 1))
                    ht = hpool.tile([P, TS], BF16, tag="ht")
                    nc.scalar.activation(out=ht, in_=ph,
                                         func=AF.Gelu_apprx_tanh,
                                         bias=b1_sb[:, m:m + 1], scale=1.0)
                    hts.append(ht)
                for tt in range(NT):
                    po = ps_o.tile([P, D], FP32, tag="po")
                    for m in range(KH):
                        nc.tensor.matmul(po,
                                         hts[m][:, tt * P:(tt + 1) * P],
                                         w2_bf[m],
                                         start=(m == 0), stop=(m == KH - 1))
                    if not b2_0:
                        nc.vector.tensor_add(out=po, in0=po, in1=B2)
                    gidx = r * NT + tt
                    gtmp = cpool.tile([P, E], FP32, tag="gtmp")
                    nc.vector.tensor_mul(out=gtmp, in0=w_sb[:, gidx, :],
                                         in1=sel_bc)
                    gt = cpool.tile([P, 1], FP32, tag="gt")
                    nc.vector.reduce_sum(out=gt, in_=gtmp, axis=AX.X)
                    part = cpool.tile([P, D], FP32, tag="part")
                    if tt % 2 == 0:
                        nc.scalar.activation(out=part, in_=po, func=AF.Copy,
                                             scale=gt)
                    else:
                        nc.vector.tensor_scalar_mul(part, po, gt)
                    nc.sync.dma_start(
                        out=partial_dram[r][tt * P:(tt + 1) * P, :], in_=part)
                nc.gpsimd.collective_compute(
                    "ReduceScatter", ALU.add, replica_groups=rg,
                    ins=[partial_dram[r].ap()],
                    outs=[rs_out_dram[r].ap()])

            # ---------------- residual + final LayerNorm ----------------
            # sqrt batched once at the end: avoids ACT table thrash
            # (sqrt<->gelu set switches) during the FFN stream.
            PF = S // N_CORES  # 64 rows per final chunk
            vars8 = ppool.tile([PF, NCHUNK], FP32, tag="vars8")
            ys, mvs = [], []
            for c in range(NCHUNK):
                y = fpool.tile([PF, D], FP32, tag="y", bufs=NCHUNK)
                nc.sync.dma_start(out=y, in_=rs_out_dram[c].ap())
                xres = fpool.tile([PF, D], FP32, tag="xres")
                nc.sync.dma_start(out=xres, in_=x_res[c, :, :])
                nc.vector.tensor_add(out=y, in0=y, in1=xres)
                stats = fpool.tile([PF, 6], FP32, tag="fstats")
                nc.vector.bn_stats(out=stats, in_=y)
                mv = fpool.tile([PF, 2], FP32, tag="fmv", bufs=NCHUNK)
                nc.vector.bn_aggr(out=mv, in_=stats)
                nc.vector.tensor_copy(vars8[:, c:c + 1], mv[:, 1:2])
                ys.append(y)
                mvs.append(mv)
            sd8 = fpool.tile([PF, NCHUNK], FP32, tag="sd8")
            rec8 = fpool.tile([PF, NCHUNK], FP32, tag="rec8")
            nc.scalar.activation(out=sd8[:, :NCHUNK - 1],
                                 in_=vars8[:, :NCHUNK - 1], func=AF.Sqrt,
                                 bias=eps_t[:PF], scale=1.0)
            nc.vector.reciprocal(out=rec8[:, :NCHUNK - 1],
                                 in_=sd8[:, :NCHUNK - 1])
            nc.scalar.activation(out=sd8[:, NCHUNK - 1:],
                                 in_=vars8[:, NCHUNK - 1:], func=AF.Sqrt,
                                 bias=eps_t[:PF], scale=1.0)
            nc.vector.reciprocal(out=rec8[:, NCHUNK - 1:],
                                 in_=sd8[:, NCHUNK - 1:])
            for c in range(NCHUNK):
                y = ys[c]
                nc.vector.tensor_scalar(out=y, in0=y, scalar1=mvs[c][:, 0:1],
                                        scalar2=rec8[:, c:c + 1],
                                        op0=ALU.subtract, op1=ALU.mult)
                if not out_g1:
                    nc.vector.tensor_mul(out=y, in0=y, in1=OG[:PF, :])
                if not out_b0:
                    nc.vector.tensor_add(out=y, in0=y, in1=OB[:PF, :])
                nc.sync.dma_start(out=out_d[c, :, :], in_=y)

    nc.finalize()
    return nc


_NC_CACHE = {}


def _get_nc(flags):
    if flags not in _NC_CACHE:
        _NC_CACHE[flags] = build(*flags)
    return _NC_CACHE[flags]


def kernel(x, Wg, W1, b1, W2, b2, ln_g, ln_b, out_g, out_b, **_run_kwargs):
    x = np.ascontiguousarray(x, dtype=np.float32)
    xf = x.reshape(T, D)
    flags = (bool(np.all(ln_g == 1)), not np.any(ln_b),
             bool(np.all(out_g == 1)), not np.any(out_b), not np.any(b2))
    nc = _get_nc(flags)
    in_maps = []
    for i in range(N_CORES):
        sel = np.zeros((E,), dtype=np.float32)
        sel[i] = 1.0
        PF = S // N_CORES
        x_res = np.stack([xf[S * c + PF * i: S * c + PF * (i + 1)]
                          for c in range(NCHUNK)])
        in_maps.append({
            "x_sh": np.ascontiguousarray(x[i]),
            "x_loc": np.ascontiguousarray(x[:NLOCAL]),
            "x_res": np.ascontiguousarray(x_res),
            "Wg": np.ascontiguousarray(Wg, dtype=np.float32),
            "W1e": np.ascontiguousarray(W1[i], dtype=np.float32),
            "b1e": np.ascontiguousarray(b1[i], dtype=np.float32),
            "W2e": np.ascontiguousarray(W2[i], dtype=np.float32),
            "b2e": np.ascontiguousarray(b2[i], dtype=np.float32),
            "ln_g": np.ascontiguousarray(ln_g, dtype=np.float32),
            "ln_b": np.ascontiguousarray(ln_b, dtype=np.float32),
            "out_g": np.ascontiguousarray(out_g, dtype=np.float32),
            "out_b": np.ascontiguousarray(out_b, dtype=np.float32),
            "sel": sel,
        })
    res = run_bass_kernel_spmd(nc, in_maps, list(range(N_CORES)),
                               **_run_kwargs)
    out = np.empty((T, D), dtype=np.float32)
    PF = S // N_CORES
    for i in range(N_CORES):
        oc = res.results[i]["out"]  # [NCHUNK, PF, D]
        for c in range(NCHUNK):
            out[S * c + PF * i: S * c + PF * (i + 1)] = oc[c]
    kernel.last_results = res
    return out.reshape(B, S, D)
